# revision 2
# baseline (speedup 1.0000x reference)
"""Trainium2 Bass kernel for nn_AttModel (masked attention GNN message passing).

Contract: kernel(**inputs) takes the FULL unsharded inputs (x [8,2048,128],
mask [8,2048,2048], Wv/Wk/Wq [128,128], bv/bk/bq [128]) and returns the full
output [8, 2048, 128] float32.

Strategy: data-parallel over batch B=8 across the 8 NeuronCores; the small
weight matrices are replicated. The device kernel runs a fully transposed
dataflow (scores computed as S^T per j-stripe) so no [N,N] transpose is ever
done on device; the host pre-transposes x, mask and the weights (pure layout
marshaling) and post-normalizes/transposes the returned outT/rowsum.

Per core (batch element b):
  qT/kT = relu(W x^T + b) as [h, n] bf16 (fp32r projections, fp32 PSUM)
  v     = relu(x W^T + b) as [j, h] bf16 (via PE transpose of vT)
  for i-chunk (1024) and j-stripe (128):
    sT = kT_j^T @ qT_chunk      (PE, bf16, PSUM fp32)
    eT = exp(sT)                (ACT, PSUM -> SBUF bf16)
    pT = eT * maskT_tile        (DVE, bf16; maskT passed as bf16 from host)
    outT_chunk   += v_j^T @ pT  (PE, accumulated in PSUM over stripes)
    rowsum_chunk += 1^T @ pT    (PE, M=1 matmul)
  Host: out_b = (outT / rowsum)^T
"""

from contextlib import ExitStack

import numpy as np
import ml_dtypes

import concourse.bass as bass
import concourse.bacc as bacc
import concourse.tile as tile
from concourse import mybir
from concourse import bass_utils

B = 8
P = 128
N = 2048
HID = 128
DIN = 128
NJ = N // P      # 16 j-stripes
ICH = 1024       # i-chunk width
NCH = N // ICH   # 2 i-chunks

f32 = mybir.dt.float32
f32r = mybir.dt.float32r
bf = mybir.dt.bfloat16
AF = mybir.ActivationFunctionType
ALU = mybir.AluOpType

_NC_CACHE = {}


def _attention_tile_kernel(ctx, tc, outT, rowsum, xT, maskT,
                           WvT, bv, WkT, bk, WqT, bq, identb):
    nc = tc.nc

    consts = ctx.enter_context(tc.tile_pool(name="consts", bufs=1))
    big = ctx.enter_context(tc.tile_pool(name="big", bufs=1))

    idb = consts.tile([P, P], bf)
    nc.sync.dma_start(out=idb, in_=identb)
    ones_col = consts.tile([P, 1], bf)
    nc.vector.memset(ones_col, 1.0)

    xT_sb = big.tile([P, N], f32r)      # [d, n]
    nc.sync.dma_start(out=xT_sb, in_=xT)
    # per-chunk tiles so downstream matmuls get fine-grained dependencies
    qTc = [big.tile([P, 512], bf, name=f"qT{c}") for c in range(4)]
    kTc = [big.tile([P, 512], bf, name=f"kT{c}") for c in range(4)]
    vNs = [big.tile([P, P], bf, name=f"vN{j}") for j in range(NJ)]

    biases = {}
    for nm, bsrc in (("q", bq), ("k", bk), ("v", bv)):
        bt = consts.tile([P, 1], f32, name=f"bias_{nm}")
        nc.sync.dma_start(out=bt, in_=bsrc)
        biases[nm] = bt

    # setup: projections (weights arrive pre-transposed [d, h])
    with tc.tile_pool(name="setup", bufs=3) as sp, \
         tc.tile_pool(name="setup_ps", bufs=2, space="PSUM") as sps:
        wTs = {}
        for nm, W in (("q", WqT), ("k", WkT), ("v", WvT)):
            wT = sp.tile([P, P], f32r, tag=f"wT_{nm}", name=f"wT_{nm}")
            nc.sync.dma_start(out=wT, in_=W)
            wTs[nm] = wT

        for nm, dest in (("q", qTc), ("k", kTc)):
            for c in range(4):
                pr_ps = sps.tile([P, 512], f32, tag="proj", name=f"proj_{nm}{c}")
                nc.tensor.matmul(pr_ps, lhsT=wTs[nm],
                                 rhs=xT_sb[:, c * 512:(c + 1) * 512],
                                 start=True, stop=True)
                nc.scalar.activation(out=dest[c], in_=pr_ps,
                                     func=AF.Relu, bias=biases[nm], scale=1.0)

        vT = sp.tile([P, N], bf, tag="vT")
        for c in range(4):
            pr_ps = sps.tile([P, 512], f32, tag="proj", name=f"proj_v{c}")
            nc.tensor.matmul(pr_ps, lhsT=wTs["v"],
                             rhs=xT_sb[:, c * 512:(c + 1) * 512],
                             start=True, stop=True)
            nc.scalar.activation(out=vT[:, c * 512:(c + 1) * 512], in_=pr_ps,
                                 func=AF.Relu, bias=biases["v"], scale=1.0)
        for jt in range(NJ):
            v_ps = sps.tile([P, P], bf, tag="vps")
            nc.tensor.transpose(v_ps, vT[:, jt * P:(jt + 1) * P], idb)
            nc.vector.tensor_copy(out=vNs[jt], in_=v_ps)

    # main loop: i-chunks x j-stripes, all in transposed score space
    mask_pool = ctx.enter_context(tc.tile_pool(name="maskp", bufs=8))
    e_pool = ctx.enter_context(tc.tile_pool(name="ep", bufs=4))
    pt_pool = ctx.enter_context(tc.tile_pool(name="ptp", bufs=4))
    out_sb_pool = ctx.enter_context(tc.tile_pool(name="outsbp", bufs=2))
    norm_pool = ctx.enter_context(tc.tile_pool(name="normp", bufs=2))
    s_psum = ctx.enter_context(tc.tile_pool(name="spsum", bufs=2, space="PSUM"))
    o_psum = ctx.enter_context(tc.tile_pool(name="opsum", bufs=1, space="PSUM"))
    r_psum = ctx.enter_context(tc.tile_pool(name="rpsum", bufs=1, space="PSUM"))

    for c in range(NCH):
        i0 = c * ICH
        o_ps = o_psum.tile([P, ICH], f32, tag="o")
        r_ps = r_psum.tile([1, ICH], f32, tag="r")
        for jt in range(NJ):
            mask_t = mask_pool.tile([P, ICH], bf, tag="mask")
            nc.sync.dma_start(out=mask_t,
                              in_=maskT[jt * P:(jt + 1) * P, i0:i0 + ICH])
            s_ps = s_psum.tile([P, ICH], f32, tag="s")
            for cc in range(2):
                icol = i0 + cc * 512
                nc.tensor.matmul(
                    s_ps[:, cc * 512:(cc + 1) * 512],
                    lhsT=kTc[jt // 4][:, (jt % 4) * P:(jt % 4 + 1) * P],
                    rhs=qTc[icol // 512],
                    start=True, stop=True)
            e_t = e_pool.tile([P, ICH], bf, tag="e")
            nc.scalar.activation(out=e_t, in_=s_ps, func=AF.Exp)
            p_t = pt_pool.tile([P, ICH], bf, tag="pt")
            nc.vector.tensor_tensor(out=p_t, in0=e_t, in1=mask_t, op=ALU.mult)
            for cc in range(2):
                nc.tensor.matmul(o_ps[:, cc * 512:(cc + 1) * 512],
                                 lhsT=vNs[jt],
                                 rhs=p_t[:, cc * 512:(cc + 1) * 512],
                                 start=(jt == 0), stop=(jt == NJ - 1))
            for cc in range(2):
                nc.tensor.matmul(r_ps[:, cc * 512:(cc + 1) * 512],
                                 lhsT=ones_col,
                                 rhs=p_t[:, cc * 512:(cc + 1) * 512],
                                 start=(jt == 0), stop=(jt == NJ - 1))

        rs_sb = norm_pool.tile([1, ICH], f32, tag="rs")
        nc.scalar.activation(out=rs_sb, in_=r_ps, func=AF.Copy)
        nc.sync.dma_start(out=rowsum[:, i0:i0 + ICH], in_=rs_sb)
        out_sb = out_sb_pool.tile([P, ICH], f32, tag="osb")
        nc.vector.tensor_copy(out=out_sb, in_=o_ps)
        nc.sync.dma_start(out=outT[:, i0:i0 + ICH], in_=out_sb)


def _build_nc():
    if "nc" in _NC_CACHE:
        return _NC_CACHE["nc"]
    nc = bacc.Bacc("TRN2", target_bir_lowering=False, debug=False, num_devices=B)
    xT = nc.dram_tensor("xT", [DIN, N], f32r, kind="ExternalInput").ap()
    maskT = nc.dram_tensor("maskT", [N, N], bf, kind="ExternalInput").ap()
    WvT = nc.dram_tensor("WvT", [DIN, HID], f32r, kind="ExternalInput").ap()
    bv = nc.dram_tensor("bv", [HID], f32, kind="ExternalInput").ap()
    WkT = nc.dram_tensor("WkT", [DIN, HID], f32r, kind="ExternalInput").ap()
    bk = nc.dram_tensor("bk", [HID], f32, kind="ExternalInput").ap()
    WqT = nc.dram_tensor("WqT", [DIN, HID], f32r, kind="ExternalInput").ap()
    bq = nc.dram_tensor("bq", [HID], f32, kind="ExternalInput").ap()
    identb = nc.dram_tensor("identb", [P, P], bf, kind="ExternalInput").ap()
    outT = nc.dram_tensor("outT", [HID, N], f32, kind="ExternalOutput").ap()
    rowsum = nc.dram_tensor("rowsum", [1, N], f32, kind="ExternalOutput").ap()

    with tile.TileContext(nc) as tc:
        with ExitStack() as ctx:
            _attention_tile_kernel(ctx, tc, outT, rowsum, xT, maskT,
                                   WvT, bv, WkT, bk, WqT, bq, identb)
    nc.compile()
    _NC_CACHE["nc"] = nc
    return nc


def build_nc():
    return _build_nc()


def make_in_maps(x, mask, Wv, bv, Wk, bk, Wq, bq):
    x = np.asarray(x, dtype=np.float32)
    mask = np.asarray(mask, dtype=np.float32)
    Wv = np.asarray(Wv, dtype=np.float32)
    bv = np.asarray(bv, dtype=np.float32)
    Wk = np.asarray(Wk, dtype=np.float32)
    bk = np.asarray(bk, dtype=np.float32)
    Wq = np.asarray(Wq, dtype=np.float32)
    bq = np.asarray(bq, dtype=np.float32)

    identb = np.eye(P, dtype=ml_dtypes.bfloat16)
    WvT = np.ascontiguousarray(Wv.T)
    WkT = np.ascontiguousarray(Wk.T)
    WqT = np.ascontiguousarray(Wq.T)
    in_maps = []
    for c in range(B):
        in_maps.append({
            "xT": np.ascontiguousarray(x[c].T),
            "maskT": np.ascontiguousarray(mask[c].T.astype(ml_dtypes.bfloat16)),
            "WvT": WvT, "bv": bv, "WkT": WkT, "bk": bk, "WqT": WqT, "bq": bq,
            "identb": identb,
        })
    return in_maps


def kernel(x, mask, Wv, bv, Wk, bk, Wq, bq):
    nc = _build_nc()
    in_maps = make_in_maps(x, mask, Wv, bv, Wk, bk, Wq, bq)
    res = bass_utils.run_bass_kernel_spmd(nc, in_maps, core_ids=list(range(B)),
                                          trace=False)
    out = np.empty((B, N, HID), dtype=np.float32)
    for c in range(B):
        outT = res.results[c]["outT"]
        rowsum = res.results[c]["rowsum"]
        rowsum = np.where(rowsum == 0.0, 1.0, rowsum)
        out[c] = (outT / rowsum).T
    return out

